# revision 1
# baseline (speedup 1.0000x reference)
"""Trainium2 Bass kernel for batched filtfilt band-pass filtering (tensorpac-style).

Math: scipy-style filtfilt with FIR taps b is (exactly) a single convolution of
the odd-extended input with the autocorrelation of b, evaluated on the interior:

    out[n] = sum_d A[d] * ext[P + n + d],   d in [-(t-1), t-1]
    A[d]   = sum_i b[i] * b[i+d]            (t = effective tap count)

provided padlen P >= t-1 (true here: P = 512, t <= 513). The left "lfilter_zi"
constant extension and the right-edge extension of the backward pass never reach
the retained [P, P+L) window, so the equivalence is exact (verified to 1e-16).

Device mapping (per core, sequence-parallel over 8 cores):
  - each core owns 2048 output positions x all 128 batches; its input is a
    (3072, 128) slice of ext^T (position-major) covering the 2x512 halo,
    shipped fp16 in the SBUF-native [partition, h-block, batch] layout.
  - out[r, (j,b)] tiles (128 positions x 4 pos-blocks x 128 batches) accumulate
    in fp32 PSUM via K=128 fp16 matmuls: lhsT = 128x128 banded-Toeplitz blocks
    of A (host-precomputed fp16 constants), rhs = 512-wide slices of ext^T.
  - per band, the number of Toeplitz blocks adapts to the true tap support
    (Q = ceil((2t+126)/128)); a half-block-shifted copy of ext^T (E64, built
    on-device from E via two partition-shifted SBUF->SBUF DMAs) lets short
    bands cover their diagonal band with Q = minimal block count.
  - loop is band-outer (large/small-Q bands interleaved) so the per-band
    constant stream (2.1 MB) overlaps the matmul phase and the PSUM drain
    stays smooth; PSUM tiles drain via a DVE/ACT split copy that also casts
    to fp16, and each band leaves as one contiguous 0.5 MB DMA on the SP
    HWDGE ring (the last band streams per group to shorten the tail).
  - dummy warm-up matmuls run while the first inputs land so the PE HAM
    clock-gate is released before real work starts.
"""

import os

import numpy as np

import concourse.mybir as mybir
from concourse import bacc
from concourse.tile import TileContext
from concourse.bass_utils import run_bass_kernel_spmd

F32 = mybir.dt.float32
F16 = mybir.dt.float16

B = 128          # batch
L = 16384        # sequence length
P = 512          # padlen (= TAPS - 1)
NB = 20          # bands
N_CORES = 8
LC = L // N_CORES            # 2048 output positions per core
GROUPS = LC // 512           # 4 groups of 512 positions
EXT_ROWS = LC + 2 * P        # 3072 ext rows per core (halo included)
H_E = EXT_ROWS // 128        # 24 aligned 128-row blocks
H_E64 = (EXT_ROWS - 128) // 128  # 23 half-shifted blocks (rows 64 + 128h + p)
N_WARM = 10                  # dummy matmuls to warm the PE HAM during input DMA

LAST_RESULT = None  # BassKernelResults of the most recent run (for test harness)

_program_cache: dict = {}


def _band_plan(kernels: np.ndarray):
    """Per-band tap support -> (t, Q, s, use64, h_base) block plan.

    Block q covers ext rows m = n0 + P - s + 128q + kk (kk = partition), so
    diagonal d = 128q + kk - s - r. Coverage of d in [-(t-1), t-1] for every
    r in [0,128) requires s >= t-1 and s <= 128Q - 127 - t. s is the smallest
    multiple of 64 >= t-1; s % 128 == 64 uses the half-shifted E64 copy.
    """
    plan = []
    for k in range(kernels.shape[0]):
        nz = np.nonzero(kernels[k])[0]
        t = int(nz[-1]) + 1 if nz.size else 1
        assert t - 1 <= P, f"band {k}: taps {t} exceed padlen {P}"
        q_cnt = (2 * t + 126 + 127) // 128
        s = 64 * ((t - 1 + 63) // 64) if t > 1 else 0
        assert s >= t - 1 and s <= 128 * q_cnt - 127 - t, (k, t, q_cnt, s)
        use64 = (s % 128) == 64
        if use64:
            h_base = (P - 64 - s) // 128
        else:
            h_base = (P - s) // 128
        assert h_base >= 0
        plan.append((t, q_cnt, s, use64, h_base))
    return plan


def _band_order(plan):
    """First a small aligned band (so PE work starts on a partial E), LAST
    the biggest band: small-Q bands drain slower than they matmul (drain
    ~1.9us/band vs Q=2 matmul 1.7us), so each must sit next to a big band
    that gives DVE/ACT slack - ending on the biggest band lets every
    earlier drain catch up and keeps the PE gap-free to the end. E64 bands
    are kept out of the first two slots to cover the E64 build latency."""
    by_q = sorted(range(len(plan)), key=lambda k: (-plan[k][1], plan[k][3]))
    small_aligned = [k for k in by_q if not plan[k][3]]
    first = small_aligned[-1] if small_aligned else by_q[0]
    last = by_q[0] if by_q[0] != first else by_q[1]
    rest = [k for k in by_q if k != first and k != last]
    # Q=2 bands matmul (1.73us) slower than they drain (1.92us): every Q=2
    # band must be followed by a Q>=3 filler or the deficit accumulates into
    # PE stalls. Interleave fillers (Q>=3, big-Q spread out) with the Q=2
    # bands one-for-one; leftover fillers go at the end before `last`.
    smalls = [k for k in rest if plan[k][1] <= 2]
    fillers = [k for k in rest if plan[k][1] >= 3]
    # spread the big-Q fillers among the Q=3 ones: big, 3, big, 3, ...
    bigs = [k for k in fillers if plan[k][1] >= 4]
    threes = [k for k in fillers if plan[k][1] == 3]
    mix = []
    while bigs or threes:
        if bigs:
            mix.append(bigs.pop(0))
        if threes:
            mix.append(threes.pop(0))
    order = [first]
    si = 0
    for f in mix:
        order.append(f)
        if si < len(smalls):
            order.append(smalls[si]); si += 1
    order.extend(smalls[si:])
    order.append(last)
    assert len(order) == len(plan) and len(set(order)) == len(plan)
    return order


def _toeplitz_blocks(kernels: np.ndarray, plan, order):
    """Stacked lhsT blocks in SBUF-native layout: (128, NBLK, 128) fp16,
    [kk, block, r] with the contraction dim kk on axis 0. Blocks are laid
    out in BAND-ORDER (slot-major) so the whole constant stream is one or
    two contiguous DMAs that land in the order the matmuls consume them."""
    nblk = sum(p[1] for p in plan)
    out = np.zeros((128, nblk, 128), np.float16)
    kk = np.arange(128)[:, None]
    rr = np.arange(128)[None, :]
    i = 0
    for k in order:
        t, q_cnt, s, _use64, _hb = plan[k]
        bk = kernels[k][:t].astype(np.float64)
        acorr = np.correlate(bk, bk, mode="full")  # length 2t-1, center t-1
        a_full = np.zeros(2 * P + 1, np.float64)
        a_full[P - (t - 1) : P + t] = acorr
        for q in range(q_cnt):
            d = 128 * q - s + kk - rr
            valid = (d >= -(t - 1)) & (d <= t - 1)
            blk = np.where(valid, a_full[np.clip(d + P, 0, 2 * P)], 0.0)
            out[:, i, :] = blk.astype(np.float16)
            i += 1
    return out


def _build_program(plan_key):
    """Compile the SPMD program for a given block structure. Cached."""
    if plan_key in _program_cache:
        return _program_cache[plan_key]

    plan = list(plan_key)
    order = _band_order(plan)
    # block offsets are SLOT-major (band-order), matching _toeplitz_blocks
    slot_offsets = np.cumsum([0] + [plan[k][1] for k in order]).tolist()
    nblk = slot_offsets[-1]
    # out-DMA taper: leading slots ship in 4-band chunks (fewer ~0.6us
    # triggers on the sequencers), the last 8 slots ship individually the
    # moment they drain (4KB/partition descriptors, alternating rings) so
    # nothing big queues at the kernel tail
    chunk_sizes = (4, 4, 4, 1, 1, 1, 1, 1, 1, 1, 1)
    assert sum(chunk_sizes) == NB
    out_chunks = []
    si = 0
    for n in chunk_sizes:
        out_chunks.append((si, n))
        si += n

    nc = bacc.Bacc("TRN2", target_bir_lowering=False, debug=False,
                   num_devices=N_CORES)
    # host-permuted ext^T slice: [p, h, b] fp16 (SBUF-native layout)
    ext_in = nc.declare_dram_parameter("ext", [128, H_E, B], F16, isOutput=False)
    lhs_in = nc.declare_dram_parameter("lhs", [128, nblk, 128], F16,
                                       isOutput=False)
    out_t = nc.declare_dram_parameter("out", [NB, 128, GROUPS * 512], F16,
                                      isOutput=True)

    with TileContext(nc) as tc:
        with (
            tc.tile_pool(name="consts", bufs=1) as cpool,
            tc.tile_pool(name="psum", bufs=8, space="PSUM") as ppool,
            tc.tile_pool(name="ostage", bufs=6) as opool,
        ):
            E = cpool.tile([128, H_E * 128], F16)
            E64 = cpool.tile([128, H_E64 * 128], F16)
            Lw = cpool.tile([128, nblk * 128], F16)
            warm = cpool.tile([128, 256], F16)
            wps = ppool.tile([128, 512], F32, tag="ps")

            # PE warm-up during the input DMAs: harmless matmuls on a zeroed
            # tile keep the HAM busy window alive so real matmuls start warm.
            # memset on DVE: nc.any would pick GpSimd, whose multi-us engine
            # cold-start delays the whole warm-up chain.
            nc.vector.memset(warm[:], 0.0)
            for w in range(N_WARM):
                nc.tensor.matmul(wps[:, 0:256], warm[:, :128], warm[:],
                                 start=True, stop=True)

            # E in 3 asymmetric chunks: the first covers exactly the h-blocks
            # the first band's g=0 matmuls touch, so real matmuls start ASAP
            t0_, q0_, _s0, _u0, hb0 = plan[order[0]]
            chunk0 = min(hb0 + q0_ + 3, 15) * 128
            e_flat = ext_in[:].rearrange("p h b -> p (h b)")
            chunk = 15 * 128
            nc.sync.dma_start(out=E[:, 0:chunk0], in_=e_flat[:, 0:chunk0])
            if chunk0 < chunk:
                nc.sync.dma_start(out=E[:, chunk0:chunk], in_=e_flat[:, chunk0:chunk])
            nc.sync.dma_start(out=E[:, chunk:], in_=e_flat[:, chunk:])
            # E64[p, h] = ext rows (64 + 128h + p), built on device from E.
            # The sem-wait of these triggers head-of-line blocks the HWDGE
            # ring, which (deliberately) gives E exclusive DMA bandwidth.
            e3 = E[:].rearrange("p (h b) -> p h b", b=B)
            e643 = E64[:].rearrange("p (h b) -> p h b", b=B)
            # flat 2D copies: (h, b) is contiguous, so each partition moves
            # as one 5.9KB run instead of per-h 256B descriptor slivers
            nc.sync.dma_start(out=E64[0:64, 0 : H_E64 * 128],
                              in_=E[64:128, 0 : H_E64 * 128])
            nc.sync.dma_start(out=E64[64:128, 0 : H_E64 * 128],
                              in_=E[0:64, 128 : (H_E64 + 1) * 128])

            # constants are pre-ordered slot-major on the host, so the 2.1 MB
            # stream is FOUR contiguous graduated DMAs on the ACT HWDGE ring
            # (4 trigger issues instead of 20). Graduation matters because a
            # DMA completes as one unit: each chunk must land before the MM
            # stream reaches its first slot, so early chunks are small.
            for lo, hi in ((0, 2), (2, 6), (6, 12), (12, NB)):
                oa, ob_ = slot_offsets[lo], slot_offsets[hi]
                nc.scalar.dma_start(
                    out=Lw[:, oa * 128 : ob_ * 128].rearrange(
                        "kk (i r) -> kk i r", r=128
                    ),
                    in_=lhs_in[:, oa:ob_, :],
                )

            # chunk tiles staged for the tapered multi-band out-DMAs
            chunk_of_slot = {}
            for ci, (s0, n) in enumerate(out_chunks):
                for j in range(n):
                    chunk_of_slot[s0 + j] = ci
            chunk_tiles = {}

            for si, k in enumerate(order):
                t, q_cnt, s, use64, h_base = plan[k]
                o = slot_offsets[si]
                src = E64 if use64 else E
                h_max = H_E64 if use64 else H_E
                ci = chunk_of_slot[si]
                s0, n = out_chunks[ci]
                if ci not in chunk_tiles:
                    chunk_tiles[ci] = opool.tile(
                        [128, n * GROUPS * 512], F16, name="obc",
                        tag=f"obc{n}", bufs=(2 if n > 1 else 3),
                    )
                ob = chunk_tiles[ci]
                obase = (si - s0) * GROUPS * 512
                if k == order[0]:
                    # group-outer for the first band only: its g=0 matmuls
                    # need just the first few E h-blocks, so PE work starts
                    # while the rest of E is still in flight
                    for g in range(GROUPS):
                        ps = ppool.tile([128, 512], F32)
                        for qi in range(q_cnt):
                            h0 = 4 * g + h_base + qi
                            assert 0 <= h0 and h0 + 4 <= h_max, (k, g, qi, h0)
                            nc.tensor.matmul(
                                ps[:],
                                Lw[:, (o + qi) * 128 : (o + qi + 1) * 128],
                                src[:, h0 * 128 : h0 * 128 + 512],
                                start=(qi == 0),
                                stop=(qi == q_cnt - 1),
                            )
                        base = obase + g * 512
                        nc.vector.tensor_copy(ob[:, base : base + 384], ps[:, 0:384])
                        nc.scalar.copy(ob[:, base + 384 : base + 512], ps[:, 384:512])
                else:
                    # qi-outer: the 4 groups' PSUM banks accumulate in
                    # lockstep so each lhsT block is (re)used by 4 back-to-
                    # back matmuls - the weight (re)load amortizes across
                    # the group sweep instead of being paid per matmul
                    pss = []
                    for g in range(GROUPS):
                        ps_g = ppool.tile([128, 512], F32, name="ps", tag="ps")
                        pss.append(ps_g)
                    for qi in range(q_cnt):
                        w = Lw[:, (o + qi) * 128 : (o + qi + 1) * 128]
                        for g in range(GROUPS):
                            h0 = 4 * g + h_base + qi
                            assert 0 <= h0 and h0 + 4 <= h_max, (k, g, qi, h0)
                            nc.tensor.matmul(
                                pss[g][:],
                                w,
                                src[:, h0 * 128 : h0 * 128 + 512],
                                start=(qi == 0),
                                stop=(qi == q_cnt - 1),
                            )
                    # split the PSUM drain across DVE and ACT so neither
                    # engine gates the PSUM bank turnaround
                    for g in range(GROUPS):
                        base = obase + g * 512
                        nc.vector.tensor_copy(ob[:, base : base + 384], pss[g][:, 0:384])
                        nc.scalar.copy(ob[:, base + 384 : base + 512], pss[g][:, 384:512])
                        if si == NB - 1:
                            # final band ships per-group on alternating rings
                            # so the kernel's last HBM completion flush is a
                            # 128KB transfer, not 512KB
                            eng = nc.sync if g % 2 == 0 else nc.scalar
                            eng.dma_start(
                                out=out_t[si, :, base : base + 512],
                                in_=ob[:, base : base + 512],
                            )
                # ship each completed chunk as ONE contiguous DMA (out_t is
                # slot-major; the host unscrambles), alternating rings
                # chunk-by-chunk. Keep the partition dim outermost on BOTH
                # sides of the AP - a leading free dim over SBUF partitions
                # generates descriptors the DGE cannot execute.
                if si == s0 + n - 1 and si != NB - 1:
                    eng = nc.sync if ci % 2 == 0 else nc.scalar
                    eng.dma_start(
                        out=out_t[s0 : s0 + n].rearrange("i p f -> p i f"),
                        in_=ob[:].rearrange("p (i f) -> p i f", i=n),
                    )

    nc.compile()
    _program_cache[plan_key] = nc
    return nc


def _maybe_register_trace_hook():
    """Best-effort registration of the axon NTFF profile hook (profiling only;
    harmless no-op if unavailable)."""
    try:
        import sys
        import types

        import antenv

        if getattr(antenv, "axon_hooks", None) is not None:
            return
        from trn_agent_boot.trn_boot import _ntff_profile_via_ctypes

        hooks = types.ModuleType("antenv.axon_hooks")
        hook = _ntff_profile_via_ctypes("/opt/axon/libaxon_pjrt.so")
        hooks.get_axon_ntff_profile_hook = lambda: hook
        hooks.set_axon_ntff_profile_hook = lambda h: None
        antenv.axon_hooks = hooks
        sys.modules["antenv.axon_hooks"] = hooks
    except Exception:
        pass


def kernel(x: np.ndarray, kernels: np.ndarray, padlen) -> np.ndarray:
    global LAST_RESULT
    x = np.asarray(x, dtype=np.float32)
    kernels = np.asarray(kernels, dtype=np.float32)
    assert x.shape == (B, 1, L) and kernels.shape[0] == NB
    assert int(padlen) == P

    plan = _band_plan(kernels)
    plan_key = tuple(plan)
    nc = _build_program(plan_key)

    order = _band_order(plan)
    lhs = np.ascontiguousarray(_toeplitz_blocks(kernels, plan, order))

    # odd extension + transpose to position-major (ext^T), fp16
    x2d = x[:, 0, :]
    left = 2.0 * x2d[:, :1] - x2d[:, 1 : P + 1][:, ::-1]
    right = 2.0 * x2d[:, -1:] - x2d[:, -P - 1 : -1][:, ::-1]
    ext_t = np.concatenate([left, x2d, right], axis=1).T.astype(np.float16)

    in_maps = []
    for c in range(N_CORES):
        sl = ext_t[c * LC : c * LC + EXT_ROWS]  # (3072, B)
        # SBUF-native layout [p, h, b]: row (128h + p) -> [p, h]
        slp = np.ascontiguousarray(
            sl.reshape(H_E, 128, B).transpose(1, 0, 2)
        )
        in_maps.append({"ext": slp, "lhs": lhs})

    trace = bool(os.environ.get("KERNEL_TRACE"))
    if trace:
        _maybe_register_trace_hook()
    res = run_bass_kernel_spmd(nc, in_maps, list(range(N_CORES)), trace=trace)
    LAST_RESULT = res

    out = np.empty((B, 1, NB, L), np.float32)
    band_of_slot = np.asarray(order)  # out rows are slot-major on device
    for c in range(N_CORES):
        dev = res.results[c]["out"].astype(np.float32).reshape(NB, 128, GROUPS, 4, 128)
        # dev[slot, r, g, j, b] -> out[b, 0, order[slot], c*LC + 512g + 128j + r]
        arr = dev.transpose(4, 0, 2, 3, 1).reshape(B, NB, LC)
        out[:, 0, band_of_slot, c * LC : (c + 1) * LC] = arr
    return out



# revision 7
# speedup vs baseline: 1.0594x; 1.0594x over previous
"""Trainium2 Bass kernel for batched filtfilt band-pass filtering (tensorpac-style).

Math: scipy-style filtfilt with FIR taps b is (exactly) a single convolution of
the odd-extended input with the autocorrelation of b, evaluated on the interior:

    out[n] = sum_d A[d] * ext[P + n + d],   d in [-(t-1), t-1]
    A[d]   = sum_i b[i] * b[i+d]            (t = effective tap count)

provided padlen P >= t-1 (true here: P = 512, t <= 513). The left "lfilter_zi"
constant extension and the right-edge extension of the backward pass never reach
the retained [P, P+L) window, so the equivalence is exact (verified to 1e-16).

A's tails are products of Hamming-window tails and decay fast: truncating to
lags |d| <= L_k with per-band tail l2 <= 3e-3 (vs the 2e-2 budget; fp16 noise
alone is 3.3e-4) shrinks the banded support. Structural gains only are taken:
the block count Q_k is fixed from the tolerance, then L_k is RAISED back to
the largest value 64*(Q_k-1) the geometry still covers, so every band keeps
the most accuracy its block count allows. This drops whole 128-blocks from
big bands (Q 9->7, 7->6, 4->3, two 3->2) and pulls the four smallest bands
under L <= 32, where FOUR bands ride in one shared 128x128 Toeplitz block
(32 output rows each, s=32): each group then needs just 4 matmuls - one per
32-position sub-offset, rhs from the E96/E/E32/E64 shifted ext copies - in
place of the 16 the four singles would need. 264 matmuls/core -> 224.

Device mapping (per core, sequence-parallel over 8 cores):
  - each core owns 2048 output positions x all 128 batches; its input is a
    (3072, 128) slice of ext^T (position-major) covering the 2x512 halo,
    shipped fp16 in the SBUF-native [partition, h-block, batch] layout.
    Eshift copies (rows 32/64/96 + 128h + p) are built on device by
    partition-shifted SBUF->SBUF DMAs spread across three HWDGE rings.
  - out tiles (128 rows x 4 pos-blocks x 128 batches) accumulate in fp32 PSUM
    via K=128 fp16 matmuls: lhsT = 128x128 banded-Toeplitz blocks of A
    (host-precomputed fp16 constants), rhs = 512-wide slices of ext^T.
  - every item runs GROUP-OUTER (PSUM drains right after each group's Q
    matmuls; LDWEIGHTS is issued per-matmul by the lowering anyway, so
    qi-outer weight amortization buys nothing). The item order interleaves
    drain-heavy items (the quad, Q=2 bands) between big-Q bands so the
    DVE/ACT drain stream never runs a deficit against the PE stream.
  - PSUM tiles drain via a DVE/ACT split copy that also casts to fp16; out
    ships in tapered multi-slot chunks (one contiguous DMA each, alternating
    rings); the final item ships per group so the kernel tail is one 128KB
    flush, not 512KB.
  - dummy warm-up matmuls run while the first inputs land so the PE HAM
    clock-gate is released before real work starts.
"""

import os

import numpy as np

import concourse.mybir as mybir
from concourse import bacc
from concourse.tile import TileContext
from concourse.bass_utils import run_bass_kernel_spmd

F32 = mybir.dt.float32
F16 = mybir.dt.float16

B = 128          # batch
L = 16384        # sequence length
P = 512          # padlen (= TAPS - 1)
NB = 20          # bands
N_CORES = 8
LC = L // N_CORES            # 2048 output positions per core
GROUPS = LC // 512           # 4 groups of 512 positions
EXT_ROWS = LC + 2 * P        # 3072 ext rows per core (halo included)
H_E = EXT_ROWS // 128        # 24 aligned 128-row blocks
H_SH = (EXT_ROWS - 128) // 128   # 23 blocks for the shifted copies
N_WARM = 4                   # dummy matmuls to warm the PE HAM during input DMA
TRUNC_TOL = 3e-3             # per-band autocorr tail l2 budget (rel)

LAST_RESULT = None  # BassKernelResults of the most recent run (for test harness)

_program_cache: dict = {}


def _acorr_full(b):
    """Autocorrelation on the full lag grid [-P, P] (float64)."""
    t = len(b)
    a = np.correlate(b, b, mode="full")  # 2t-1, center t-1
    a_full = np.zeros(2 * P + 1, np.float64)
    a_full[P - (t - 1): P + t] = a
    return a_full


def _band_plan(kernels: np.ndarray):
    """Per-band truncated lag support L and block geometry.

    Block q covers ext rows m = n0 + P - s + 128q + kk (kk = partition), so
    diagonal d = 128q + kk - s - r. Coverage of d in [-L, L] for every
    r in [0,128) requires s >= L and s <= 128Q - 128 - L; s is a multiple
    of 64 (s % 128 == 64 sources the rhs from the 64-shifted ext copy).
    Q is fixed from the truncation tolerance, then L raised to 64*(Q-1),
    the largest lag the Q-block geometry covers. Bands whose tolerance
    support is <= 32 are quadable: four bands share one block at 32 output
    rows each (s = 32, d = kk - 32 - r' in [-63, 95] covers |d| <= 32).
    """
    plan = []
    for k in range(kernels.shape[0]):
        nz = np.nonzero(kernels[k])[0]
        t = int(nz[-1]) + 1 if nz.size else 1
        assert t - 1 <= P, f"band {k}: taps {t} exceed padlen {P}"
        b = kernels[k][:t].astype(np.float64)
        a = np.correlate(b, b, mode="full")
        c0 = t - 1
        nrm = np.linalg.norm(a) + 1e-300
        L_min = t - 1
        for Ltry in range(t - 2, -1, -1):
            tail = np.concatenate([a[: c0 - Ltry], a[c0 + Ltry + 1:]])
            if np.linalg.norm(tail) / nrm <= TRUNC_TOL:
                L_min = Ltry
            else:
                break
        quadable = L_min <= 32
        if quadable:
            Lv = min(t - 1, 32)
            s, q = 32, 1
        else:
            s_min = 64 * ((L_min + 63) // 64) if L_min > 0 else 0
            q = (s_min + L_min + 128 + 127) // 128
            Lv = min(t - 1, 64 * (q - 1))
            s = 64 * ((Lv + 63) // 64) if Lv > 0 else 0
            assert s >= Lv and s <= 128 * q - 128 - Lv, (k, Lv, s, q)
        use64 = (s % 128) == 64
        h_base = (P - 64 - s) // 128 if use64 else (P - s) // 128
        assert h_base >= 0
        plan.append((t, Lv, q, s, use64, h_base, quadable))
    # quads hold exactly 4 bands; demote leftovers to plain Q=2 singles
    quadbands = [k for k in range(len(plan)) if plan[k][6]]
    for k in quadbands[4 * (len(quadbands) // 4):]:
        t = plan[k][0]
        plan[k] = (t, min(t - 1, 64), 2, 64, True, (P - 128) // 128, False)
    return plan


def _build_items(plan):
    """Group bands into schedule items (normal bands and 32-row quads) and
    order them so the DVE/ACT drain stream keeps pace with the PE stream.

    Drain model (per 2048-col slot: ~1.86us; per-slot matmul: Q*4*216ns):
    a quad produces 4 slots off 16 matmuls (slack -4us), Q=2 bands -0.13us,
    Q>=4 bands +1.6..+4.2us. Start on an aligned (E-only) Q=3 band (the
    shifted ext copies land a few us into the matmul stream), interleave
    bigs with Q=2 bands, park the quad after the third big so its sources
    (built over three DMA rings after E lands) are ready, and end on an
    aligned Q=3 band whose groups ship individually."""
    quadbands = [k for k in range(len(plan)) if plan[k][6]]
    normals = [k for k in range(len(plan)) if not plan[k][6]]
    items = []
    assert len(quadbands) % 4 == 0  # _band_plan demoted any leftovers
    for qi in range(0, len(quadbands), 4):
        items.append({"kind": "quad", "bands": tuple(quadbands[qi: qi + 4]),
                      "nslots": 4, "nblk": 1})
    for k in normals:
        items.append({"kind": "normal", "band": k, "nslots": 1,
                      "nblk": plan[k][2]})

    def q_of(it):
        return plan[it["band"]][2] if it["kind"] == "normal" else 0

    def aligned(it):
        return it["kind"] == "normal" and not plan[it["band"]][4]

    q3s = sorted([it for it in items if it["kind"] == "normal"
                  and q_of(it) == 3 and aligned(it)],
                 key=lambda it: -plan[it["band"]][1])
    assert len(q3s) >= 2, "need aligned Q=3 bands for first/last"
    first, last = q3s[0], q3s[1]
    rest = [it for it in items if it is not first and it is not last]
    bigs = sorted([it for it in rest if it["kind"] == "normal" and q_of(it) >= 4],
                  key=lambda it: -q_of(it))
    quads = [it for it in rest if it["kind"] == "quad"]
    q2s = [it for it in rest if it["kind"] == "normal" and q_of(it) == 2]
    mids = [it for it in rest if it["kind"] == "normal" and q_of(it) == 3]
    order = [first]
    li = 0
    for bi, bg in enumerate(bigs):
        order.append(bg)
        if bi >= 2 and quads:
            order.append(quads.pop(0))
        elif li < len(q2s):
            order.append(q2s[li]); li += 1
    order.extend(quads)
    for md in mids:
        if li < len(q2s):
            order.append(q2s[li]); li += 1
        order.append(md)
    order.extend(q2s[li:])
    order.append(last)
    assert len(order) == len(items)
    so = bo = 0
    for it in order:
        it["slot"] = so
        it["block_off"] = bo
        so += it["nslots"]
        bo += it["nblk"]
    return order, so, bo


def _toeplitz_blocks(kernels: np.ndarray, plan, items, nblk):
    """Stacked lhsT blocks in SBUF-native layout: (128, NBLK, 128) fp16,
    [kk, block, r] with the contraction dim kk on axis 0, laid out in
    schedule order so the constant stream is a few contiguous DMAs."""
    out = np.zeros((128, nblk, 128), np.float16)
    kk = np.arange(128)[:, None]

    def banded(k, dmat):
        t, Lv = plan[k][0], plan[k][1]
        a_full = _acorr_full(kernels[k][:t].astype(np.float64))
        valid = (dmat >= -Lv) & (dmat <= Lv)
        return np.where(valid, a_full[np.clip(dmat + P, 0, 2 * P)], 0.0)

    for it in items:
        o = it["block_off"]
        if it["kind"] == "normal":
            k = it["band"]
            s = plan[k][3]
            rr = np.arange(128)[None, :]
            for q in range(it["nblk"]):
                d = 128 * q - s + kk - rr
                out[:, o + q, :] = banded(k, d).astype(np.float16)
        else:
            blk = np.zeros((128, 128))
            rq = np.arange(32)[None, :]
            for i, k in enumerate(it["bands"]):
                blk[:, 32 * i: 32 * i + 32] = banded(k, kk - 32 - rq)
            out[:, o, :] = blk.astype(np.float16)
    return out


def _out_chunks(items):
    """Tapered out-DMA chunking over schedule items: leading items group into
    ~2-slot chunks (fewer ~0.6us triggers; a quad ships as its own 4-slot
    chunk), trailing items ship solo the moment they drain; the last item
    ships per-group inside the main loop."""
    n = len(items)
    chunks = []
    cur = []
    cur_slots = 0
    for idx, it in enumerate(items[:-1]):
        if it["kind"] == "quad":
            if cur:
                chunks.append(cur)
            chunks.append([idx])
            cur, cur_slots = [], 0
            continue
        solo_zone = idx >= n - 6
        cur.append(idx)
        cur_slots += it["nslots"]
        if solo_zone or cur_slots >= 2:
            chunks.append(cur)
            cur, cur_slots = [], 0
    if cur:
        chunks.append(cur)
    chunks.append([n - 1])  # final item: per-group ship
    return chunks


def _build_program(plan_key):
    """Compile the SPMD program for a given block structure. Cached."""
    if plan_key in _program_cache:
        return _program_cache[plan_key]

    plan = list(plan_key)
    items, nslots, nblk = _build_items(plan)
    assert nslots == NB
    chunks = _out_chunks(items)
    chunk_of_item = {}
    for ci, idxs in enumerate(chunks):
        for idx in idxs:
            chunk_of_item[idx] = ci

    # lhs constant stream graduation (item-range boundaries -> block ranges)
    n_it = len(items)
    lhs_cuts = sorted({0, 1, 2, min(4, n_it), min(7, n_it), n_it})

    nc = bacc.Bacc("TRN2", target_bir_lowering=False, debug=False,
                   num_devices=N_CORES)
    # host-permuted ext^T slice: [p, h, b] fp16 (SBUF-native layout)
    ext_in = nc.declare_dram_parameter("ext", [128, H_E, B], F16, isOutput=False)
    lhs_in = nc.declare_dram_parameter("lhs", [128, nblk, 128], F16,
                                       isOutput=False)
    out_t = nc.declare_dram_parameter("out", [NB, 128, GROUPS * 512], F16,
                                      isOutput=True)

    need_quad = any(it["kind"] == "quad" for it in items)

    with TileContext(nc) as tc:
        with (
            tc.tile_pool(name="consts", bufs=1) as cpool,
            tc.tile_pool(name="psum", bufs=8, space="PSUM") as ppool,
            tc.tile_pool(name="ostage", bufs=6) as opool,
        ):
            E = cpool.tile([128, H_E * 128], F16)
            E64 = cpool.tile([128, H_SH * 128], F16)
            if need_quad:
                E32 = cpool.tile([128, H_SH * 128], F16)
                E96 = cpool.tile([128, H_SH * 128], F16)
            Lw = cpool.tile([128, nblk * 128], F16)
            warm = cpool.tile([128, 256], F16)
            wps = ppool.tile([128, 512], F32, tag="ps")

            # PE warm-up during the input DMAs: harmless matmuls on a zeroed
            # tile keep the HAM busy window alive so real matmuls start warm.
            # memset on DVE: nc.any would pick GpSimd, whose multi-us engine
            # cold-start delays the whole warm-up chain.
            nc.vector.memset(warm[:], 0.0)
            for w in range(N_WARM):
                nc.tensor.matmul(wps[:, 0:256], warm[:, :128], warm[:],
                                 start=True, stop=True)

            # E in 2 chunks: the first covers the h-blocks the first two
            # items' g=0 matmuls touch (each chunk costs ~128 descriptor
            # issues regardless of width, so fewer chunks finish sooner)
            e_flat = ext_in[:].rearrange("p h b -> p (h b)")
            chunk0 = 12 * 128
            nc.sync.dma_start(out=E[:, 0:chunk0], in_=e_flat[:, 0:chunk0])
            nc.sync.dma_start(out=E[:, chunk0:], in_=e_flat[:, chunk0:])
            # Shifted ext copies Es[p, h] = ext rows (s + 128h + p), built on
            # device from E by partition-shifted flat 2D copies ((h, b) is
            # contiguous, so each partition moves as one ~5.9KB run). Their
            # sem-waits head-of-line block their HWDGE rings (only SP and ACT
            # exist): E64 then E32 on SP right after E (E64 feeds slot ~2,
            # E32 only the mid-schedule quad), E96 on ACT after the lhs
            # stream so the constants are never stuck behind a build.
            nf = H_SH * 128
            nc.sync.dma_start(out=E64[0:64, 0:nf], in_=E[64:128, 0:nf])
            nc.sync.dma_start(out=E64[64:128, 0:nf], in_=E[0:64, 128: 128 + nf])
            if need_quad:
                nc.sync.dma_start(out=E32[0:96, 0:nf], in_=E[32:128, 0:nf])
                nc.sync.dma_start(out=E32[96:128, 0:nf],
                                  in_=E[0:32, 128: 128 + nf])

            # constants are pre-ordered schedule-major on the host, so the
            # ~1.3 MB stream is a few contiguous graduated DMAs on the ACT
            # HWDGE ring. Graduation matters because a DMA completes as one
            # unit: each chunk must land before the MM stream reaches its
            # first block, so early chunks are small.
            for lo, hi in zip(lhs_cuts[:-1], lhs_cuts[1:]):
                oa = items[lo]["block_off"]
                ob_ = (items[hi]["block_off"] if hi < n_it else nblk)
                nc.scalar.dma_start(
                    out=Lw[:, oa * 128: ob_ * 128].rearrange(
                        "kk (i r) -> kk i r", r=128
                    ),
                    in_=lhs_in[:, oa:ob_, :],
                )
            if need_quad:
                nc.scalar.dma_start(out=E96[0:32, 0:nf], in_=E[96:128, 0:nf])
                nc.scalar.dma_start(out=E96[32:128, 0:nf],
                                    in_=E[0:96, 128: 128 + nf])

            # staging tiles for the tapered multi-slot out-DMAs
            chunk_tiles = {}
            chunk_slot0 = {}
            for ci, idxs in enumerate(chunks):
                ns = sum(items[idx]["nslots"] for idx in idxs)
                chunk_slot0[ci] = items[idxs[0]]["slot"]
                chunk_tiles[ci] = opool.tile(
                    [128, ns * GROUPS * 512], F16, name="obc",
                    tag=f"obc{ns}", bufs=(2 if ns > 1 else 3),
                )

            def drain(ps, ob, base):
                # split the PSUM drain across DVE and ACT so neither engine
                # gates the PSUM bank turnaround
                nc.vector.tensor_copy(ob[:, base: base + 384], ps[:, 0:384])
                nc.scalar.copy(ob[:, base + 384: base + 512], ps[:, 384:512])

            last_idx = len(items) - 1
            for idx, it in enumerate(items):
                ci = chunk_of_item[idx]
                ob = chunk_tiles[ci]
                obase = (it["slot"] - chunk_slot0[ci]) * GROUPS * 512
                o = it["block_off"]
                if it["kind"] == "normal":
                    k = it["band"]
                    _t, _L, q_cnt, _s, use64, h_base, _qd = plan[k]
                    src = E64 if use64 else E
                    h_max = H_SH if use64 else H_E
                    for g in range(GROUPS):
                        ps = ppool.tile([128, 512], F32, name="ps", tag="ps")
                        for qq in range(q_cnt):
                            h0 = 4 * g + h_base + qq
                            assert 0 <= h0 and h0 + 4 <= h_max, (k, g, qq, h0)
                            nc.tensor.matmul(
                                ps[:],
                                Lw[:, (o + qq) * 128: (o + qq + 1) * 128],
                                src[:, h0 * 128: h0 * 128 + 512],
                                start=(qq == 0),
                                stop=(qq == q_cnt - 1),
                            )
                        base = obase + g * 512
                        drain(ps, ob, base)
                        if idx == last_idx:
                            # final item ships per-group on alternating rings
                            # so the kernel's last HBM flush is 128KB
                            eng = nc.sync if g % 2 == 0 else nc.scalar
                            eng.dma_start(
                                out=out_t[it["slot"], :, g * 512: g * 512 + 512],
                                in_=ob[:, base: base + 512],
                            )
                else:
                    # quad: one shared lhsT block, 4 bands x 32 rows; four
                    # matmuls per group, one per 32-position sub-offset,
                    # rhs from the four shifted ext copies (s = 32)
                    w = Lw[:, o * 128: (o + 1) * 128]
                    srcs = ((E96, 3), (E, 4), (E32, 4), (E64, 4))
                    for g in range(GROUPS):
                        for ss, (src, hb) in enumerate(srcs):
                            h0 = hb + 4 * g
                            ps = ppool.tile([128, 512], F32, name="ps", tag="ps")
                            nc.tensor.matmul(ps[:], w,
                                             src[:, h0 * 128: h0 * 128 + 512],
                                             start=True, stop=True)
                            drain(ps, ob, obase + ss * GROUPS * 512 + g * 512)
                # ship each completed chunk as ONE contiguous DMA (out_t is
                # slot-major; the host unscrambles), alternating rings
                # chunk-by-chunk. Keep the partition dim outermost on BOTH
                # sides of the AP - a leading free dim over SBUF partitions
                # generates descriptors the DGE cannot execute.
                if idx == chunks[ci][-1] and idx != last_idx:
                    s0 = chunk_slot0[ci]
                    ns = sum(items[j]["nslots"] for j in chunks[ci])
                    eng = nc.sync if ci % 2 == 0 else nc.scalar
                    eng.dma_start(
                        out=out_t[s0: s0 + ns].rearrange("i p f -> p i f"),
                        in_=ob[:].rearrange("p (i f) -> p i f", i=ns),
                    )

    nc.compile()
    _program_cache[plan_key] = (nc, items)
    return nc, items


def _maybe_register_trace_hook():
    """Best-effort registration of the axon NTFF profile hook (profiling only;
    harmless no-op if unavailable)."""
    try:
        import sys
        import types

        import antenv

        if getattr(antenv, "axon_hooks", None) is not None:
            return
        from trn_agent_boot.trn_boot import _ntff_profile_via_ctypes

        hooks = types.ModuleType("antenv.axon_hooks")
        hook = _ntff_profile_via_ctypes("/opt/axon/libaxon_pjrt.so")
        hooks.get_axon_ntff_profile_hook = lambda: hook
        hooks.set_axon_ntff_profile_hook = lambda h: None
        antenv.axon_hooks = hooks
        sys.modules["antenv.axon_hooks"] = hooks
    except Exception:
        pass


def kernel(x: np.ndarray, kernels: np.ndarray, padlen) -> np.ndarray:
    global LAST_RESULT
    x = np.asarray(x, dtype=np.float32)
    kernels = np.asarray(kernels, dtype=np.float32)
    assert x.shape == (B, 1, L) and kernels.shape[0] == NB
    assert int(padlen) == P

    plan = _band_plan(kernels)
    plan_key = tuple(plan)
    nc, items = _build_program(plan_key)

    nblk = sum(it["nblk"] for it in items)
    lhs = np.ascontiguousarray(_toeplitz_blocks(kernels, plan, items, nblk))

    # odd extension + transpose to position-major (ext^T), fp16
    x2d = x[:, 0, :]
    left = 2.0 * x2d[:, :1] - x2d[:, 1: P + 1][:, ::-1]
    right = 2.0 * x2d[:, -1:] - x2d[:, -P - 1: -1][:, ::-1]
    ext_t = np.concatenate([left, x2d, right], axis=1).T.astype(np.float16)

    in_maps = []
    for c in range(N_CORES):
        sl = ext_t[c * LC: c * LC + EXT_ROWS]  # (3072, B)
        # SBUF-native layout [p, h, b]: row (128h + p) -> [p, h]
        slp = np.ascontiguousarray(
            sl.reshape(H_E, 128, B).transpose(1, 0, 2)
        )
        in_maps.append({"ext": slp, "lhs": lhs})

    trace = bool(os.environ.get("KERNEL_TRACE"))
    if trace:
        _maybe_register_trace_hook()
    res = run_bass_kernel_spmd(nc, in_maps, list(range(N_CORES)), trace=trace)
    LAST_RESULT = res

    out = np.empty((B, 1, NB, L), np.float32)
    for c in range(N_CORES):
        dev = res.results[c]["out"].astype(np.float32)
        dev = dev.reshape(NB, 128, GROUPS, 4, 128)  # [slot, r, g, j, b]
        cl = slice(c * LC, (c + 1) * LC)
        for it in items:
            s = it["slot"]
            if it["kind"] == "normal":
                # dev[s, r, g, j, b] -> out[b, 0, k, c*LC + 512g + 128j + r]
                out[:, 0, it["band"], cl] = (
                    dev[s].transpose(3, 1, 2, 0).reshape(B, LC)
                )
            else:
                # slot s+ss = sub-offset ss; rows 32i:32i+32 = band i of the
                # quad; position = 512g + 128j + 32*ss + r'
                quad = dev[s: s + 4].reshape(4, 4, 32, GROUPS, 4, 128)
                # [ss, i, r', g, j, b] -> [i, b, g, j, ss, r']
                quad = quad.transpose(1, 5, 3, 4, 0, 2).reshape(4, B, LC)
                for i, k in enumerate(it["bands"]):
                    out[:, 0, k, cl] = quad[i]
    return out
